# revision 37
# baseline (speedup 1.0000x reference)
"""Trainium2 Bass kernel for the soft-logic-gate (moe_routing) problem.

Math (per output column j):
    nw = softmax(weights[j, :16]); W = nw @ COEFF  (4 affine coeffs)
    out[:, j] = W0 + Wa * x[:, idx_a[j]] + Wb * x[:, idx_b[j]] + Wab * a * b

Strategy: shard the 16384 output columns across 8 NeuronCores (2048 each).
Host passes x transposed and quantized to uint8 (xT [in_dim, batch]; x is
uniform in [0,1), so u8 costs ~0.2% rel err). The device gathers rows with
two wide multi-index indirect DMAs per input (8 rows per partition per
call), casting u8->fp16 in the SDMA stream; this amortizes the ~1us fixed
SWDGE descriptor-generation cost that dominated the per-block gather
variant. Output columns sit on partitions, so the per-column coefficients
are natural per-partition scalars for tensor_scalar/activation ops.

The result is computed in fp16 scaled so r' = 252*r + 1.5 lies in
[~1.5, ~253.5] (margin for fp16/quant noise at the u8 edges) and stored
with a SWDGE cast-store fp16->u8 (halving HBM write traffic); the host
unscales while unsharding.  All dequant/output scales are folded into the
COEFF constants host-side.

The softmax->coeff fold runs on device in fp32 as before. Compute per
block: t = Wab*a + Wb, v = Wa*a + W0, u = b*t, r = u + v; v always runs on
the scalar engine, t alternates ACT/DVE to balance engine load, u/r on DVE.
"""

import json
from contextlib import ExitStack

import numpy as np

from concourse import bass, mybir, tile


def _split_multiwait_bir(bir_json: bytes) -> bytes:
    """Rewrite BIR so no compute instruction carries more than one sem wait.

    The walrus build in this container rejects >1 embedded sync wait per
    compute instruction ("Too many sync wait commands"), but the Tile
    scheduler emits them. Hoist extra waits onto standalone EventSemaphore
    instructions inserted immediately before, on the same engine (waits are
    AND conditions, so splitting preserves semantics and per-engine order).
    """
    b = json.loads(bir_json)
    counter = 0
    for f in b.get("functions", []):
        for blk in f.get("blocks", []):
            ins = blk.get("instructions")
            if not ins:
                continue
            out = []
            for inst in ins:
                si = inst.get("sync_info") or {}
                waits = si.get("on_wait") or []
                if len(waits) > 1 and "engine" in inst:
                    for w in waits[:-1]:
                        counter += 1
                        out.append(
                            {
                                "debug": inst.get("debug", 0),
                                "engine": inst["engine"],
                                "ins": [],
                                "outs": [],
                                "name": f"evw_{counter}_{inst['name']}",
                                "opcode": "EventSemaphore",
                                "sync_info": {"on_update": [], "on_wait": [w]},
                            }
                        )
                    si["on_wait"] = [waits[-1]]
                out.append(inst)
            blk["instructions"] = out
    return json.dumps(b).encode()


def _install_bir_patch():
    import concourse.bass_utils as _bu
    import concourse.bass2jax as _b2j

    if getattr(_bu, "_multiwait_patch", False):
        return
    orig = _bu.compile_bir_kernel

    def patched(bir_json, tmpdir, neff_name="file.neff"):
        return orig(_split_multiwait_bir(bir_json), tmpdir, neff_name)

    _bu.compile_bir_kernel = patched
    _b2j.compile_bir_kernel = patched
    _bu._multiwait_patch = True


_install_bir_patch()

F32 = mybir.dt.float32
F16 = mybir.dt.float16
U8 = mybir.dt.uint8
I32 = mybir.dt.int32
P = 128

# COEFF[g] = [k0, ka, kb, kab]: gate_g(a,b) = k0 + ka*a + kb*b + kab*a*b
COEFF = np.array(
    [
        [0, 0, 0, 0], [0, 0, 0, 1], [0, 1, 0, -1], [0, 1, 0, 0],
        [0, 0, 1, -1], [0, 0, 1, 0], [0, 1, 1, -2], [0, 1, 1, -1],
        [1, -1, -1, 1], [1, -1, -1, 2], [1, 0, -1, 0], [1, 0, -1, 1],
        [1, -1, 0, 0], [1, -1, 0, 1], [1, 0, 0, -1], [1, 0, 0, 0],
    ],
    dtype=np.float32,
)  # [16 gates, 4 coeffs]

NG = 16  # number of gates
OUT_SCALE = 252.0  # r' = OUT_SCALE*r + OUT_BIAS stored as u8
OUT_BIAS = 1.5
CHUNK = 4  # blocks per gather/store chunk


def build_nc(B, IN, SH, num_devices=8):
    """Build the per-core Bass program.

    B: batch size, IN: in_dim, SH: output-column shard per core.
    DRAM tensors (per core): xT [IN, B] u8 (replicated), idx [P, 2*nblk] i32
    (idx[p, blk] = idx_a[blk*P+p], idx[p, nblk+blk] = idx_b[...]),
    wc [P, 5*nblk*NG] f32 (wq then cfr as in make_core_inputs),
    outT [SH, B] u8.
    """
    nblk = SH // P
    assert SH % P == 0 and B % P == 0 and nblk % CHUNK == 0
    nch = nblk // CHUNK

    nc = bass.Bass("TRN2", debug=False, num_devices=num_devices,
                  dynamic_dma_scratch_size=49152)
    xT = nc.dram_tensor("xT", [IN, B], U8, kind="ExternalInput").ap()
    idx = nc.dram_tensor("idx", [P, 2 * nblk], I32, kind="ExternalInput").ap()
    wc = nc.dram_tensor("wc", [P, 5 * nblk * NG], F32, kind="ExternalInput").ap()
    outT = nc.dram_tensor("outT", [SH, B], F16, kind="ExternalOutput").ap()

    mult, add = mybir.AluOpType.mult, mybir.AluOpType.add
    AF = mybir.ActivationFunctionType

    with tile.TileContext(nc) as tc, ExitStack() as ctx:
        consts = ctx.enter_context(tc.tile_pool(name="consts", bufs=1))
        wpool = ctx.enter_context(tc.tile_pool(name="w", bufs=1))
        gpool = ctx.enter_context(tc.tile_pool(name="gather", bufs=10))
        tpool = ctx.enter_context(tc.tile_pool(name="temps", bufs=3))
        opool = ctx.enter_context(tc.tile_pool(name="outs", bufs=6))

        idx_t = consts.tile([P, 2 * nblk], I32)
        nc.sync.dma_start(idx_t[:], idx)
        wc_t = consts.tile([P, 5 * nblk * NG], F32)
        nc.sync.dma_start(wc_t[:], wc)

        # Per-column effective coefficients W [P, 4*nblk]; W[:, k*nblk+blk]
        # is coeff k for output columns j = blk*P + p.
        W = consts.tile([P, 4 * nblk], F32)
        esum = consts.tile([P, nblk], F32)
        rsum = consts.tile([P, nblk], F32)

        e = wpool.tile([P, nblk * NG], F32)
        nc.scalar.activation(e[:], wc_t[:, :nblk * NG], AF.Exp)
        e3 = e[:].rearrange("p (n g) -> p n g", g=NG)
        nc.vector.reduce_sum(out=esum[:], in_=e3, axis=mybir.AxisListType.X)
        nc.vector.reciprocal(rsum[:], esum[:])
        for k in range(4):
            scr = wpool.tile([P, nblk * NG], F32, tag="scr")
            nc.vector.tensor_tensor(
                out=scr[:],
                in0=e[:],
                in1=wc_t[:, (1 + k) * nblk * NG:(2 + k) * nblk * NG],
                op=mult,
            )
            nc.vector.reduce_sum(
                out=W[:, k * nblk:(k + 1) * nblk],
                in_=scr[:].rearrange("p (n g) -> p n g", g=NG),
                axis=mybir.AxisListType.X,
            )
            nc.vector.tensor_tensor(
                out=W[:, k * nblk:(k + 1) * nblk],
                in0=W[:, k * nblk:(k + 1) * nblk],
                in1=rsum[:],
                op=mult,
            )

        for c in range(nch):
            for j in range(CHUNK):
                blk = c * CHUNK + j
                # Single-index cast-gathers (multi-index + cast is
                # miscompiled by this toolchain; verified on HW).
                a_t = gpool.tile([P, B], F16, tag="a")
                nc.gpsimd.indirect_dma_start(
                    out=a_t[:],
                    out_offset=None,
                    in_=xT,
                    in_offset=bass.IndirectOffsetOnAxis(
                        ap=idx_t[:, blk:blk + 1], axis=0
                    ),
                )
                b_t = gpool.tile([P, B], F16, tag="b")
                nc.gpsimd.indirect_dma_start(
                    out=b_t[:],
                    out_offset=None,
                    in_=xT,
                    in_offset=bass.IndirectOffsetOnAxis(
                        ap=idx_t[:, nblk + blk:nblk + blk + 1], axis=0
                    ),
                )
                W0 = W[:, 0 * nblk + blk: 0 * nblk + blk + 1]
                Wa = W[:, 1 * nblk + blk: 1 * nblk + blk + 1]
                Wb = W[:, 2 * nblk + blk: 2 * nblk + blk + 1]
                Wab = W[:, 3 * nblk + blk: 3 * nblk + blk + 1]

                t_c = tpool.tile([P, B], F16, tag="t")
                v_c = tpool.tile([P, B], F16, tag="v")
                u_c = tpool.tile([P, B], F16, tag="u")
                r_c = opool.tile([P, B], F16, tag="r")
                # v always on ACT; t on ACT for 1/3 of blocks, else DVE
                # (tensor_scalar runs 4x on fp16; scalar_tensor_tensor
                # would run 1x and slow buffer recycling). x=6 t's on ACT
                # balances ACT (16+6)*2.09 ~= DVE 10*0.81+32*1.22.
                nc.scalar.activation(v_c[:], a_t[:], AF.Identity, bias=W0, scale=Wa)
                if blk % 3 == 0:
                    nc.scalar.activation(t_c[:], a_t[:], AF.Identity, bias=Wb, scale=Wab)
                else:
                    nc.vector.tensor_scalar(
                        out=t_c[:], in0=a_t[:], scalar1=Wab, scalar2=Wb,
                        op0=mult, op1=add,
                    )
                # Last block in half-width chunks so the kernel tail
                # (post-last-gather dependency chain) is shorter.
                chunks = [slice(0, B)] if blk < nblk - 1 else [
                    slice(0, B // 2), slice(B // 2, B)
                ]
                for cs in chunks:
                    nc.vector.tensor_tensor(
                        out=u_c[:, cs], in0=b_t[:, cs], in1=t_c[:, cs], op=mult)
                    nc.vector.tensor_tensor(
                        out=r_c[:, cs], in0=u_c[:, cs], in1=v_c[:, cs], op=add)
                    # HWDGE store (separate stream from the SWDGE gathers).
                    nc.sync.dma_start(outT[blk * P:(blk + 1) * P, cs], r_c[:, cs])
    return nc


def make_core_inputs(x, weights, idx_a, idx_b, n_cores):
    """Host-side shard prep. Returns (in_maps, SH)."""
    B, IN = x.shape
    OUT = weights.shape[0]
    SH = OUT // n_cores
    nblk = SH // P
    # x in [0,1) is shipped as uint8 (the SWDGE gather casts u8->fp16 on the
    # fly); the 1/255 dequant scales and the 252/+1.5 output quantization are
    # folded into the COEFF constants:
    #   r' = 252*(W0 + Wa*a + Wb*b + Wab*ab) + 1.5, a = a8/255 etc.
    xT = np.ascontiguousarray(
        np.rint(np.asarray(x, dtype=np.float32).T * 255.0).astype(np.uint8)
    )
    cq = COEFF.copy() * OUT_SCALE
    cq[:, 0] += OUT_BIAS  # sums against softmax weights (sum=1) -> +OUT_BIAS
    cq[:, 1] /= 255.0
    cq[:, 2] /= 255.0
    cq[:, 3] /= 255.0 * 255.0
    # cfr[p, k*nblk*NG + blk*NG + g] = cq[g, k], replicated over p and blk
    cfr = np.ascontiguousarray(
        np.broadcast_to(
            np.repeat(cq.T.reshape(4, 1, NG), nblk, axis=1).reshape(
                1, 4 * nblk * NG
            ),
            (P, 4 * nblk * NG),
        )
    ).astype(np.float32)
    idx_a = np.asarray(idx_a)
    idx_b = np.asarray(idx_b)
    weights = np.asarray(weights, dtype=np.float32)
    in_maps = []
    perms = []
    for c in range(n_cores):
        sl = slice(c * SH, (c + 1) * SH)
        # Sort the shard's columns by idx_a so each a-gather call reads
        # ascending (HBM row-buffer friendly) rows; undone on unshard.
        perm = np.argsort(idx_a[sl], kind="stable").astype(np.int64)
        perms.append(perm)
        ias = idx_a[sl][perm]
        ibs = idx_b[sl][perm]
        ws = weights[sl][perm]
        ia = ias.reshape(nblk, P).T.astype(np.int32)
        ib = ibs.reshape(nblk, P).T.astype(np.int32)
        idxc = np.ascontiguousarray(np.concatenate([ia, ib], axis=1))
        # wq[p, blk*NG+g] = ws[blk*P + p, g]
        wqc = ws.reshape(nblk, P, NG).transpose(1, 0, 2).reshape(P, nblk * NG)
        wcc = np.ascontiguousarray(np.concatenate([wqc, cfr], axis=1))
        in_maps.append({"xT": xT, "wc": wcc, "idx": idxc})
    return in_maps, SH, perms


_CACHE = {}

N_CORES = 8


def _get_nc(B, IN, SH):
    key = (B, IN, SH)
    if key not in _CACHE:
        _CACHE[key] = build_nc(B, IN, SH, num_devices=N_CORES)
    return _CACHE[key]


def kernel(x, weights, idx_a, idx_b, _trace=False, _tmpdir=None):
    """Full-input entry point: returns out [batch, out_dim] float32."""
    from concourse.bass_utils import run_bass_kernel_spmd

    x = np.asarray(x, dtype=np.float32)
    B, IN = x.shape
    in_maps, SH, perms = make_core_inputs(x, weights, idx_a, idx_b, N_CORES)
    nc = _get_nc(B, IN, SH)
    kw = {}
    if _trace:
        kw = {"trace": True, "tmpdir": _tmpdir}
    res = run_bass_kernel_spmd(nc, in_maps, core_ids=list(range(N_CORES)), **kw)
    full = np.empty((B, SH * N_CORES), dtype=np.float32)
    for c in range(N_CORES):
        r8 = res.results[c]["outT"].astype(np.float32)
        # dequant, sorted column order -> unsorted
        full[:, c * SH + perms[c]] = ((r8 - OUT_BIAS) / OUT_SCALE).T
    if _trace:
        kernel.last_results = res
    return full
